# revision 15
# baseline (speedup 1.0000x reference)
"""BasicTransformer Trainium2 kernel (Bass/Tile), data-parallel over batch on 8 cores.

Per batch b (one NeuronCore each), all matmul operands fp16 (fp32 PSUM accum):
    M   = W_q^T @ W_k                 (512,512)  once per core, f32r
    Wt  = (lin_w @ W_v)^T             (512,512)  once per core (folds the
                                      post-attention Linear into the V path)
    e   = embed[x]                    (T, D)     indirect-DMA gather
    G   = e @ M                       ([d,t] layout)  so scores = G e^T = q^T k
    Vt  = e @ Wt                      ([t,a] layout)  = v @ lin_w^T
    S   = G^T-slices . E              PE -> PSUM [128, T] per 128-query chunk
    p   = exp(S*scale - rowmax)       DVE rowmax + ACT exp (accum -> l), fp16
    z   = relu((p^T-transposed @ Vt) / l + lin_b)   PE + DVE scale + ACT relu,
                                      accumulated over t per 512-block
    out = sigmoid(clf_w . mean + clf_b)

t-order inside the kernel is a fixed permutation of the true t-order; the
computation is permutation-invariant over t (softmax over keys, p@v
contraction, mean over t), so the final (1,) output is unaffected.
"""

import math
import os

import numpy as np

B, T, D, VOCAB = 8, 2048, 512, 32000
P = 128
TC = T // P          # 16 t-chunks
DC = D // P          # 4 d-chunks
NB = T // 512        # 4 blocks of 512 along t
SCALE = 1.0 / math.sqrt(D)
N_CORES = 8

_COMPILED = {}


def _build(iters=1, mm_dtype=None):
    import concourse.bacc as bacc
    import concourse.mybir as mybir
    import concourse.tile as tile
    from concourse.masks import make_identity

    dt = mybir.dt

    nc = bacc.Bacc("TRN2", target_bir_lowering=False, debug=False)

    x_d = nc.declare_dram_parameter("x", [T], dt.int32, isOutput=False)
    emb_d = nc.declare_dram_parameter("embed", [VOCAB + 1, D], dt.float32, isOutput=False)
    wq_d = nc.declare_dram_parameter("W_q", [D, D], dt.float32r, isOutput=False)
    wk_d = nc.declare_dram_parameter("W_k", [D, D], dt.float32r, isOutput=False)
    wv_d = nc.declare_dram_parameter("W_v", [D, D], dt.float32r, isOutput=False)
    lw_d = nc.declare_dram_parameter("lin_w", [D, D], dt.float32r, isOutput=False)
    lb_d = nc.declare_dram_parameter("lin_b", [D], dt.float32, isOutput=False)
    cw_d = nc.declare_dram_parameter("clf_w", [D], dt.float32, isOutput=False)
    cb_d = nc.declare_dram_parameter("clf_b", [1], dt.float32, isOutput=False)
    out_d = nc.declare_dram_parameter("out", [iters, 1], dt.float32, isOutput=True)

    with tile.TileContext(nc) as tc:
        with tc.tile_pool(name="const", bufs=1) as cpool:
            ident = cpool.tile([P, P], dt.float32, tag="ident", name="ident")
            make_identity(nc, ident[:])
            identr = cpool.tile([P, P], dt.float32r, tag="identr", name="identr")
            nc.vector.tensor_copy(identr[:], ident[:])
            ident16 = cpool.tile([P, P], dt.float16, tag="ident16", name="ident16")
            nc.vector.tensor_copy(ident16[:], ident[:])

            for it in range(iters):
                _body(nc, tc, mybir, dt, (identr, ident16),
                      x_d, emb_d, wq_d, wk_d, wv_d, lw_d, lb_d, cw_d, cb_d,
                      out_d.ap()[it:it + 1, :])

    nc.compile()
    return nc


def _body(nc, tc, mybir, dt, idents,
          x_d, emb_d, wq_d, wk_d, wv_d, lw_d, lb_d, cw_d, cb_d, out_ap):
    import concourse.bass as bass

    identr, ident16 = idents

    AF = mybir.ActivationFunctionType
    AX = mybir.AxisListType
    ALU = mybir.AluOpType

    # alternate DVE / ACT for PSUM->SBUF copies to balance engine load
    _cp = [0]

    def copy_ps(out, in_):
        if _cp[0] % 2 == 0:
            nc.vector.tensor_copy(out, in_)
        else:
            nc.scalar.copy(out, in_)
        _cp[0] += 1

    with tc.tile_pool(name="persist", bufs=1) as pp:
        E16 = pp.tile([P, DC, T], dt.float16, tag="e16", name="e16")
        G16 = pp.tile([P, DC, T], dt.float16, tag="g16", name="g16")
        V16 = pp.tile([P, TC, 512], dt.float16, tag="v16", name="v16")
        M16 = pp.tile([P, DC, D], dt.float16, tag="m16", name="m16")
        Wt16 = pp.tile([P, DC, D], dt.float16, tag="wt16", name="wt16")
        Lbc = pp.tile([P, T], dt.float16, tag="lbc", name="lbc")
        Linv = pp.tile([P, TC], dt.float32, tag="linv", name="linv")
        Linv16 = pp.tile([P, TC], dt.float16, tag="linv16", name="linv16")
        linb = pp.tile([P, DC], dt.float32, tag="linb", name="linb")
        clfw = pp.tile([P, DC], dt.float32, tag="clfw", name="clfw")
        clfb = pp.tile([1, 1], dt.float32, tag="clfb", name="clfb")
        zsum = [pp.tile([P, NB], dt.float32, tag=f"zs{d}", name=f"zs{d}") for d in range(DC)]

        nc.sync.dma_start(out=linb[:], in_=lb_d.ap().rearrange("(c p) -> p c", p=P))
        nc.sync.dma_start(out=clfw[:], in_=cw_d.ap().rearrange("(c p) -> p c", p=P))
        nc.sync.dma_start(out=clfb[:], in_=cb_d.ap().unsqueeze(1))

        # ---------------- setup: M, Wt, gather+transpose, G, Vt ------------
        with tc.tile_pool(name="wsb", bufs=1) as wp, \
             tc.tile_pool(name="etf_pool", bufs=8) as efp, \
             tc.tile_pool(name="e16_pool", bufs=5) as e16p, \
             tc.tile_pool(name="setup_ps", bufs=2, space="PSUM") as spp, \
             tc.tile_pool(name="gv_ps", bufs=2, space="PSUM") as gvp:

            idx = wp.tile([P, TC], dt.int32, tag="idx", name="idx")
            nc.sync.dma_start(out=idx[:], in_=x_d.ap().rearrange("(p c) -> p c", c=TC))

            wq = wp.tile([P, DC, D], dt.float32r, tag="wq", name="wq")
            wk = wp.tile([P, DC, D], dt.float32r, tag="wk", name="wk")
            wv = wp.tile([P, DC, D], dt.float32r, tag="wv", name="wv")
            lw = wp.tile([P, DC, D], dt.float32r, tag="lw", name="lw")
            # chunked so the M matmuls can start after the first 512KB lands
            for dc in range(DC):
                for w_t, w_d in ((wq, wq_d), (wk, wk_d)):
                    nc.sync.dma_start(
                        out=w_t[:, dc, :],
                        in_=w_d.ap()[dc * P:(dc + 1) * P, :])
            for w_t, w_d in ((wv, wv_d), (lw, lw_d)):
                nc.sync.dma_start(out=w_t[:],
                                  in_=w_d.ap().rearrange("(c p) m -> p c m", p=P))

            # first gather group issued before the PE weight-precompute so the
            # embedding rows stream in behind the weight DMAs
            def gather(g):
                tiles = [efp.tile([P, D], dt.float32, tag="etf", name="etf")
                         for _ in range(4)]
                for s in range(4):
                    nc.gpsimd.indirect_dma_start(
                        out=tiles[s][:],
                        out_offset=None,
                        in_=emb_d.ap(),
                        in_offset=bass.IndirectOffsetOnAxis(
                            ap=idx[:, g * 4 + s:g * 4 + s + 1], axis=0),
                    )
                return tiles

            pending = [gather(0)]

            # M = W_q^T @ W_k   ([f,g], fp16)
            for fc in range(DC):
                mp = spp.tile([P, D], dt.float32, tag="mps", name="mps")
                for dc in range(DC):
                    nc.tensor.matmul(mp[:], wq[:, dc, fc * P:(fc + 1) * P],
                                     wk[:, dc, :], start=(dc == 0),
                                     stop=(dc == DC - 1))
                copy_ps(M16[:, fc, :], mp[:])

            # lin_w^T ([d2,a], f32r) then Wt = (lin_w @ W_v)^T ([d1,a], fp16)
            def do_wt():
                lwT = wp.tile([P, DC, D], dt.float32r, tag="lwT", name="lwT")
                for c in range(DC):
                    tp = spp.tile([P, 4, P], dt.float32r, tag="mps", name="wtp")
                    for s in range(4):
                        nc.tensor.transpose(tp[:, s, :], lw[:, s, c * P:(c + 1) * P],
                                            identr[:])
                    copy_ps(lwT[:, c, :], tp[:])
                for c in range(DC):
                    wtp = spp.tile([P, D], dt.float32, tag="mps", name="wtps")
                    for d2 in range(DC):
                        nc.tensor.matmul(wtp[:], wv[:, d2, c * P:(c + 1) * P],
                                         lwT[:, d2, :], start=(d2 == 0),
                                         stop=(d2 == DC - 1))
                    copy_ps(Wt16[:, c, :], wtp[:])

            # per gather group: cast fp16, transpose into E16, then G and Vt
            def e_transpose(g):
                etf = pending.pop()
                e16t = [e16p.tile([P, D], dt.float16, tag="e16t", name="e16t")
                        for _ in range(4)]
                for s in range(4):
                    nc.vector.tensor_copy(e16t[s][:], etf[s][:])
                if g + 1 < 4:
                    pending.append(gather(g + 1))
                for fc in range(DC):
                    tp = spp.tile([P, 4, P], dt.float16, tag="etp", name="etp")
                    for s in range(4):
                        nc.tensor.transpose(tp[:, s, :],
                                            e16t[s][:, fc * P:(fc + 1) * P],
                                            ident16[:])
                    copy_ps(E16[:, fc, g * 512:(g + 1) * 512], tp[:])

            def gv_group(g):
                # G slice for this t-block
                for gc in range(DC):
                    gps = gvp.tile([P, 512], dt.float32, tag="gps", name="gps")
                    for fc in range(DC):
                        nc.tensor.matmul(gps[:], M16[:, fc, gc * P:(gc + 1) * P],
                                         E16[:, fc, g * 512:(g + 1) * 512],
                                         start=(fc == 0), stop=(fc == DC - 1))
                    copy_ps(G16[:, gc, g * 512:(g + 1) * 512], gps[:])
                # Vt chunks for this t-block
                for s in range(4):
                    c = g * 4 + s
                    vps = gvp.tile([P, 512], dt.float32, tag="gps", name="vps")
                    for d1 in range(DC):
                        nc.tensor.matmul(vps[:], E16[:, d1, c * P:(c + 1) * P],
                                         Wt16[:, d1, :], start=(d1 == 0),
                                         stop=(d1 == DC - 1))
                    copy_ps(V16[:, c, :], vps[:])

            e_transpose(0)
            do_wt()
            gv_group(0)
            for g in range(1, 4):
                e_transpose(g)
                gv_group(g)

        # ---------------- attention + folded linear + mean ----------------
        with tc.tile_pool(name="pbuf", bufs=2) as ppb, \
             tc.tile_pool(name="pt_sb", bufs=2) as ptp, \
             tc.tile_pool(name="s_ps", bufs=1, space="PSUM") as sps, \
             tc.tile_pool(name="t_ps", bufs=2, space="PSUM") as tps, \
             tc.tile_pool(name="z_ps", bufs=2, space="PSUM") as zps, \
             tc.tile_pool(name="scratch", bufs=2) as scr:

            zjunk = scr.tile([P, 512], dt.float16, tag="zjunk", name="zjunk", bufs=1)

            state = {}

            def stage_scores(ic):
                S = sps.tile([P, T], dt.float32, tag="s", name="s")
                mx4 = scr.tile([P, NB], dt.float32, tag="mx4", name="mx4")
                for jb in range(NB):
                    for gc in range(DC):
                        nc.tensor.matmul(S[:, jb * 512:(jb + 1) * 512],
                                         G16[:, gc, ic * P:(ic + 1) * P],
                                         E16[:, gc, jb * 512:(jb + 1) * 512],
                                         start=(gc == 0), stop=(gc == DC - 1),
                                         skip_group_check=True)
                    nc.vector.tensor_reduce(mx4[:, jb:jb + 1],
                                            S[:, jb * 512:(jb + 1) * 512],
                                            axis=AX.X, op=ALU.max)
                state[ic] = (S, mx4)

            def stage_softmax(ic):
                S, mx4 = state.pop(ic)
                mx = scr.tile([P, 1], dt.float32, tag="mx", name="mx")
                nc.vector.tensor_reduce(mx[:], mx4[:], axis=AX.X, op=ALU.max)
                negb = scr.tile([P, 1], dt.float32, tag="negb", name="negb")
                nc.vector.tensor_scalar_mul(negb[:], mx[:], -float(SCALE))
                lp = scr.tile([P, 1], dt.float32, tag="lp", name="lp")
                Pex = ppb.tile([P, T], dt.float16, tag="pex", name="pex")
                nc.scalar.activation(Pex[:], S[:], AF.Exp,
                                     bias=negb[:], scale=float(SCALE),
                                     accum_out=lp[:])
                state[ic] = (Pex, lp)

            def stage_transpose(ic, PT):
                Pex, lp = state.pop(ic)
                nc.vector.reciprocal(Linv[:, ic:ic + 1], lp[:])
                nc.vector.tensor_copy(Linv16[:, ic:ic + 1], Linv[:, ic:ic + 1])
                s_i = ic % 4
                for g in range(TC // 4):
                    tp = tps.tile([P, 4, P], dt.float16, tag="tp", name="tp")
                    for s in range(4):
                        jc = g * 4 + s
                        nc.tensor.transpose(tp[:, s, :],
                                            Pex[:, jc * P:(jc + 1) * P], ident16[:])
                    # DVE only: keeps the ACT queue clear for the next exp
                    nc.vector.tensor_copy(
                        PT[:, g * 4:(g + 1) * 4, s_i * P:(s_i + 1) * P], tp[:])
                # broadcast 1/l along the free dim for the block-level scale
                lt = tps.tile([P, 4, P], dt.float16, tag="tp", name="lt")
                nc.tensor.transpose(lt[:, 0, :],
                                    Linv16[:, ic:ic + 1].to_broadcast([P, P]),
                                    ident16[:])
                nc.scalar.copy(Lbc[:, ic * P:(ic + 1) * P], lt[:, 0, :])

            def stage_pv_chunk(bo, dc, PT):
                zp = zps.tile([P, 512], dt.float32, tag="zp", name="zp")
                for jc in range(TC):
                    nc.tensor.matmul(zp[:], V16[:, jc, dc * P:(dc + 1) * P],
                                     PT[:, jc, :],
                                     start=(jc == 0), stop=(jc == TC - 1))
                zr = scr.tile([P, 512], dt.float16, tag="zr", name="zr")
                nc.vector.tensor_tensor(out=zr[:], in0=zp[:],
                                        in1=Lbc[:, bo * 512:(bo + 1) * 512],
                                        op=ALU.mult)
                nc.scalar.activation(zjunk[:], zr[:], AF.Relu,
                                     bias=linb[:, dc:dc + 1], scale=1.0,
                                     accum_out=zsum[dc][:, bo:bo + 1])

            # software pipeline: scores(ic+1) is emitted after PT(ic-1) and a
            # PV chunk of the previous block, so exp(ic) hides under PE work
            PTs = {}
            stage_scores(0)
            stage_softmax(0)
            for ic in range(TC):
                bo = ic // 4
                if ic % 4 == 0:
                    PTs[bo] = ptp.tile([P, TC, 512], dt.float16, tag="pt", name="pt")
                if ic + 1 < TC:
                    stage_scores(ic + 1)
                    stage_softmax(ic + 1)
                stage_transpose(ic, PTs[bo])
                if bo > 0:
                    stage_pv_chunk(bo - 1, ic % 4, PTs[bo - 1])
                if ic % 4 == 3 and bo > 0:
                    PTs.pop(bo - 1)
            for dc in range(DC):
                stage_pv_chunk(NB - 1, dc, PTs[NB - 1])

            # ---------------- classifier ----------------
            ysum = [scr.tile([P, 1], dt.float32, tag=f"ys{d}", name=f"ys{d}")
                    for d in range(DC)]
            for dc in range(DC):
                nc.vector.tensor_reduce(ysum[dc][:], zsum[dc][:], axis=AX.X,
                                        op=ALU.add)
            op = tps.tile([P, 4, P], dt.float32, tag="tp", name="optp")
            for dc in range(DC):
                nc.tensor.matmul(op[:1, 0, :1], clfw[:, dc:dc + 1], ysum[dc][:],
                                 start=(dc == 0), stop=(dc == DC - 1))
            osb = scr.tile([1, 1], dt.float32, tag="osb", name="osb")
            nc.scalar.activation(osb[:], op[:1, 0, :1], AF.Sigmoid,
                                 bias=clfb[:], scale=float(1.0 / T))
            nc.sync.dma_start(out=out_ap, in_=osb[:])


def _get_nc(iters=1, mm_dtype=None):
    key = (iters,)
    if key not in _COMPILED:
        _COMPILED[key] = _build(iters=iters)
    return _COMPILED[key]


def _in_maps(x, embed, W_q, W_k, W_v, lin_w, lin_b, clf_w, clf_b):
    x = np.ascontiguousarray(np.asarray(x).astype(np.int32))
    common = {
        "embed": np.ascontiguousarray(np.asarray(embed, np.float32)),
        "W_q": np.ascontiguousarray(np.asarray(W_q, np.float32)),
        "W_k": np.ascontiguousarray(np.asarray(W_k, np.float32)),
        "W_v": np.ascontiguousarray(np.asarray(W_v, np.float32)),
        "lin_w": np.ascontiguousarray(np.asarray(lin_w, np.float32)),
        "lin_b": np.ascontiguousarray(np.asarray(lin_b, np.float32).reshape(D)),
        "clf_w": np.ascontiguousarray(np.asarray(clf_w, np.float32).reshape(D)),
        "clf_b": np.ascontiguousarray(np.asarray(clf_b, np.float32).reshape(1)),
    }
    return [dict(common, x=x[c]) for c in range(N_CORES)]


def kernel(x, embed, W_q, W_k, W_v, lin_w, lin_b, clf_w, clf_b):
    from concourse.bass_utils import run_bass_kernel_spmd

    nc = _get_nc()
    in_maps = _in_maps(x, embed, W_q, W_k, W_v, lin_w, lin_b, clf_w, clf_b)
    res = run_bass_kernel_spmd(nc, in_maps, core_ids=list(range(N_CORES)))
    out = np.stack([res.results[c]["out"][0, 0] for c in range(N_CORES)])
    return out.reshape(B, 1).astype(np.float32)


# revision 26
# speedup vs baseline: 1.2398x; 1.2398x over previous
"""BasicTransformer Trainium2 kernel (Bass/Tile), data-parallel over batch on 8 cores.

Per batch b (one NeuronCore each), all matmul operands fp16 (fp32 PSUM accum):
    M   = W_q^T @ W_k                 (512,512)  once per core, f32r
    Wt  = (lin_w @ W_v)^T             (512,512)  once per core (folds the
                                      post-attention Linear into the V path)
    e   = embed[x]                    (T, D)     indirect-DMA gather
    G   = e @ M                       ([d,t] layout)  so scores = G e^T = q^T k
    Vt  = e @ Wt                      ([t,a] layout)  = v @ lin_w^T
    S   = G^T-slices . E              PE -> PSUM [128, T] per 128-query chunk
    p   = exp(S*scale - rowmax)       DVE rowmax + ACT exp (accum -> l), fp16
    z   = relu((p^T-transposed @ Vt) / l + lin_b)   PE + DVE scale + ACT relu,
                                      accumulated over t per 512-block
    out = sigmoid(clf_w . mean + clf_b)

t-order inside the kernel is a fixed permutation of the true t-order; the
computation is permutation-invariant over t (softmax over keys, p@v
contraction, mean over t), so the final (1,) output is unaffected.
"""

import math
import os

import numpy as np

B, T, D, VOCAB = 8, 2048, 512, 32000
P = 128
TC = T // P          # 16 t-chunks
DC = D // P          # 4 d-chunks
NB = T // 512        # 4 blocks of 512 along t
SCALE = 1.0 / math.sqrt(D)
N_CORES = 8

_COMPILED = {}


def _build(iters=1, mm_dtype=None):
    import concourse.bacc as bacc
    import concourse.mybir as mybir
    import concourse.tile as tile
    from concourse.masks import make_identity

    dt = mybir.dt

    nc = bacc.Bacc("TRN2", target_bir_lowering=False, debug=False)

    x_d = nc.declare_dram_parameter("x", [T], dt.int32, isOutput=False)
    emb_d = nc.declare_dram_parameter("embed", [VOCAB + 1, D], dt.float32, isOutput=False)
    wq_d = nc.declare_dram_parameter("W_q", [D, D], dt.float32r, isOutput=False)
    wk_d = nc.declare_dram_parameter("W_k", [D, D], dt.float32r, isOutput=False)
    wv_d = nc.declare_dram_parameter("W_v", [D, D], dt.float32r, isOutput=False)
    lw_d = nc.declare_dram_parameter("lin_w", [D, D], dt.float32r, isOutput=False)
    lb_d = nc.declare_dram_parameter("lin_b", [D], dt.float32, isOutput=False)
    cw_d = nc.declare_dram_parameter("clf_w", [D], dt.float32, isOutput=False)
    cb_d = nc.declare_dram_parameter("clf_b", [1], dt.float32, isOutput=False)
    out_d = nc.declare_dram_parameter("out", [iters, 1], dt.float32, isOutput=True)

    with tile.TileContext(nc) as tc:
        with tc.tile_pool(name="const", bufs=1) as cpool:
            ident = cpool.tile([P, P], dt.float32, tag="ident", name="ident")
            make_identity(nc, ident[:])
            identr = cpool.tile([P, P], dt.float32r, tag="identr", name="identr")
            nc.vector.tensor_copy(identr[:], ident[:])
            ident16 = cpool.tile([P, P], dt.float16, tag="ident16", name="ident16")
            nc.vector.tensor_copy(ident16[:], ident[:])

            carry = {}
            for it in range(iters):
                _body(nc, tc, mybir, dt, (identr, ident16),
                      x_d, emb_d, wq_d, wk_d, wv_d, lw_d, lb_d, cw_d, cb_d,
                      out_d.ap()[it:it + 1, :], carry)

    nc.compile()
    return nc


def _body(nc, tc, mybir, dt, idents,
          x_d, emb_d, wq_d, wk_d, wv_d, lw_d, lb_d, cw_d, cb_d, out_ap, carry):
    import concourse.bass as bass
    import bass_rust

    identr, ident16 = idents

    AF = mybir.ActivationFunctionType
    AX = mybir.AxisListType
    ALU = mybir.AluOpType

    def dep(winst, rinst, why):
        bass_rust.add_dep_helper(winst.ins, rinst.ins, sync=True, reason=why)

    class Ring:
        """Explicit WAR guards for PSUM tile-pool rings: the tile framework
        only chains ring reuse on the previous writer, not its readers."""

        def __init__(self, bufs):
            self.bufs = bufs
            self.readers = []
            self.n = 0

        def writer(self, winst):
            j = self.n - self.bufs
            self.n += 1
            if 0 <= j < len(self.readers) and self.readers[j] is not None:
                dep(winst, self.readers[j], "ring WAR")
            return winst

        def reader(self, rinst):
            self.readers.append(rinst)
            return rinst

    # alternate DVE / ACT for PSUM->SBUF copies to balance engine load
    _cp = [0]

    def copy_ps(out, in_):
        if _cp[0] % 2 == 0:
            r = nc.vector.tensor_copy(out, in_)
        else:
            r = nc.scalar.copy(out, in_)
        _cp[0] += 1
        return r

    with tc.tile_pool(name="persist", bufs=1) as pp:
        E16 = pp.tile([P, DC, T], dt.float16, tag="e16", name="e16")
        G16 = pp.tile([P, DC, T], dt.float16, tag="g16", name="g16")
        V16 = pp.tile([P, TC, 512], dt.float16, tag="v16", name="v16")
        M16 = pp.tile([P, DC, D], dt.float16, tag="m16", name="m16")
        Wt16 = pp.tile([P, DC, D], dt.float16, tag="wt16", name="wt16")
        Lbc = pp.tile([P, T], dt.float16, tag="lbc", name="lbc")
        Linv = pp.tile([P, TC], dt.float32, tag="linv", name="linv")
        Linv16 = pp.tile([P, TC], dt.float16, tag="linv16", name="linv16")
        linb = pp.tile([P, DC], dt.float32, tag="linb", name="linb")
        clfw = pp.tile([P, DC], dt.float32, tag="clfw", name="clfw")
        clfb = pp.tile([1, 1], dt.float32, tag="clfb", name="clfb")
        zsum = [pp.tile([P, NB], dt.float32, tag=f"zs{d}", name=f"zs{d}") for d in range(DC)]

        nc.sync.dma_start(out=linb[:], in_=lb_d.ap().rearrange("(c p) -> p c", p=P))
        nc.sync.dma_start(out=clfw[:], in_=cw_d.ap().rearrange("(c p) -> p c", p=P))
        nc.sync.dma_start(out=clfb[:], in_=cb_d.ap().unsqueeze(1))

        # ---------------- setup: M, Wt, gather+transpose, G, Vt ------------
        with tc.tile_pool(name="wsb", bufs=1) as wp, \
             tc.tile_pool(name="etf_pool", bufs=8) as efp, \
             tc.tile_pool(name="e16_pool", bufs=5) as e16p, \
             tc.tile_pool(name="setup_ps", bufs=2, space="PSUM") as spp, \
             tc.tile_pool(name="gv_ps", bufs=2, space="PSUM") as gvp:

            idx = wp.tile([P, TC], dt.int32, tag="idx", name="idx")
            nc.sync.dma_start(out=idx[:], in_=x_d.ap().rearrange("(p c) -> p c", c=TC))

            wq = wp.tile([P, DC, D], dt.float32r, tag="wq", name="wq")
            wk = wp.tile([P, DC, D], dt.float32r, tag="wk", name="wk")
            wv = wp.tile([P, DC, D], dt.float32r, tag="wv", name="wv")
            lw = wp.tile([P, DC, D], dt.float32r, tag="lw", name="lw")
            # chunked so the M matmuls can start after the first 512KB lands
            for dc in range(DC):
                for w_t, w_d in ((wq, wq_d), (wk, wk_d)):
                    nc.sync.dma_start(
                        out=w_t[:, dc, :],
                        in_=w_d.ap()[dc * P:(dc + 1) * P, :])
            for w_t, w_d in ((wv, wv_d), (lw, lw_d)):
                nc.sync.dma_start(out=w_t[:],
                                  in_=w_d.ap().rearrange("(c p) m -> p c m", p=P))

            # first gather group issued before the PE weight-precompute so the
            # embedding rows stream in behind the weight DMAs
            def gather(g):
                tiles = [efp.tile([P, D], dt.float32, tag="etf", name="etf")
                         for _ in range(4)]
                for s in range(4):
                    nc.gpsimd.indirect_dma_start(
                        out=tiles[s][:],
                        out_offset=None,
                        in_=emb_d.ap(),
                        in_offset=bass.IndirectOffsetOnAxis(
                            ap=idx[:, g * 4 + s:g * 4 + s + 1], axis=0),
                    )
                return tiles

            pending = [gather(0)]

            mps_ring = Ring(2)
            etp_ring = Ring(2)
            gps_ring = Ring(2)

            # M = W_q^T @ W_k   ([f,g], fp16)
            for fc in range(DC):
                mp = spp.tile([P, D], dt.float32, tag="mps", name="mps")
                for dc in range(DC):
                    w = nc.tensor.matmul(mp[:], wq[:, dc, fc * P:(fc + 1) * P],
                                         wk[:, dc, :], start=(dc == 0),
                                         stop=(dc == DC - 1))
                    if dc == 0:
                        mps_ring.writer(w)
                        if fc == 0 and "prev_end" in carry:
                            dep(w, carry["prev_end"], "cross-iter PSUM")
                mps_ring.reader(copy_ps(M16[:, fc, :], mp[:]))

            # lin_w^T ([d2,a], f32r) then Wt = (lin_w @ W_v)^T ([d1,a], fp16)
            def do_wt():
                lwT = wp.tile([P, DC, D], dt.float32r, tag="lwT", name="lwT")
                for c in range(DC):
                    tp = spp.tile([P, 4, P], dt.float32r, tag="mps", name="wtp")
                    for s in range(4):
                        w = nc.tensor.transpose(tp[:, s, :],
                                                lw[:, s, c * P:(c + 1) * P],
                                                identr[:])
                        if s == 0:
                            mps_ring.writer(w)
                    mps_ring.reader(copy_ps(lwT[:, c, :], tp[:]))
                for c in range(DC):
                    wtp = spp.tile([P, D], dt.float32, tag="mps", name="wtps")
                    for d2 in range(DC):
                        w = nc.tensor.matmul(wtp[:], wv[:, d2, c * P:(c + 1) * P],
                                             lwT[:, d2, :], start=(d2 == 0),
                                             stop=(d2 == DC - 1))
                        if d2 == 0:
                            mps_ring.writer(w)
                    mps_ring.reader(copy_ps(Wt16[:, c, :], wtp[:]))

            # per gather group: cast fp16, transpose into E16, then G and Vt
            def e_transpose(g):
                etf = pending.pop()
                e16t = [e16p.tile([P, D], dt.float16, tag="e16t", name="e16t")
                        for _ in range(4)]
                for s in range(4):
                    nc.vector.tensor_copy(e16t[s][:], etf[s][:])
                if g + 1 < 4:
                    pending.append(gather(g + 1))
                for fc in range(DC):
                    tp = spp.tile([P, 4, P], dt.float16, tag="etp", name="etp")
                    for s in range(4):
                        w = nc.tensor.transpose(tp[:, s, :],
                                                e16t[s][:, fc * P:(fc + 1) * P],
                                                ident16[:])
                        if s == 0:
                            etp_ring.writer(w)
                    etp_ring.reader(copy_ps(E16[:, fc, g * 512:(g + 1) * 512], tp[:]))

            def gv_group(g):
                # G slice for this t-block
                for gc in range(DC):
                    gps = gvp.tile([P, 512], dt.float32, tag="gps", name="gps")
                    for fc in range(DC):
                        w = nc.tensor.matmul(gps[:], M16[:, fc, gc * P:(gc + 1) * P],
                                             E16[:, fc, g * 512:(g + 1) * 512],
                                             start=(fc == 0), stop=(fc == DC - 1))
                        if fc == 0:
                            gps_ring.writer(w)
                    gps_ring.reader(copy_ps(G16[:, gc, g * 512:(g + 1) * 512], gps[:]))
                # Vt chunks for this t-block
                for s in range(4):
                    c = g * 4 + s
                    vps = gvp.tile([P, 512], dt.float32, tag="gps", name="vps")
                    for d1 in range(DC):
                        w = nc.tensor.matmul(vps[:], E16[:, d1, c * P:(c + 1) * P],
                                             Wt16[:, d1, :], start=(d1 == 0),
                                             stop=(d1 == DC - 1))
                        if d1 == 0:
                            gps_ring.writer(w)
                    gps_ring.reader(copy_ps(V16[:, c, :], vps[:]))

            e_transpose(0)
            do_wt()
            gv_group(0)
            for g in range(1, 4):
                e_transpose(g)
                gv_group(g)

        # ---------------- attention + folded linear + mean ----------------
        with tc.tile_pool(name="pbuf", bufs=2) as ppb, \
             tc.tile_pool(name="pt_sb", bufs=2) as ptp, \
             tc.tile_pool(name="s_ps", bufs=1, space="PSUM") as sps, \
             tc.tile_pool(name="t_ps", bufs=2, space="PSUM") as tps, \
             tc.tile_pool(name="z_ps", bufs=2, space="PSUM") as zps, \
             tc.tile_pool(name="scratch", bufs=2) as scr:

            zjunk = scr.tile([P, 512], dt.float16, tag="zjunk", name="zjunk", bufs=1)

            state = {}
            s_ring = Ring(1)
            tp_ring = Ring(2)
            zp_ring = Ring(2)

            def stage_scores(ic):
                S = sps.tile([P, T], dt.float32, tag="s", name="s")
                mx4 = scr.tile([P, NB], dt.float32, tag="mx4", name="mx4")
                for jb in range(NB):
                    for gc in range(DC):
                        w = nc.tensor.matmul(S[:, jb * 512:(jb + 1) * 512],
                                             G16[:, gc, ic * P:(ic + 1) * P],
                                             E16[:, gc, jb * 512:(jb + 1) * 512],
                                             start=(gc == 0), stop=(gc == DC - 1),
                                             skip_group_check=True)
                        if jb == 0 and gc == 0:
                            s_ring.writer(w)
                    nc.vector.tensor_reduce(mx4[:, jb:jb + 1],
                                            S[:, jb * 512:(jb + 1) * 512],
                                            axis=AX.X, op=ALU.max)
                state[ic] = (S, mx4)

            def stage_softmax(ic):
                S, mx4 = state.pop(ic)
                mx = scr.tile([P, 1], dt.float32, tag="mx", name="mx")
                nc.vector.tensor_reduce(mx[:], mx4[:], axis=AX.X, op=ALU.max)
                negb = scr.tile([P, 1], dt.float32, tag="negb", name="negb")
                nc.vector.tensor_scalar_mul(negb[:], mx[:], -float(SCALE))
                lp = scr.tile([P, 1], dt.float32, tag="lp", name="lp")
                Pex = ppb.tile([P, T], dt.float16, tag="pex", name="pex")
                s_ring.reader(nc.scalar.activation(Pex[:], S[:], AF.Exp,
                                                   bias=negb[:], scale=float(SCALE),
                                                   accum_out=lp[:]))
                state[ic] = (Pex, lp)

            def stage_transpose(ic, PT):
                Pex, lp = state.pop(ic)
                nc.vector.reciprocal(Linv[:, ic:ic + 1], lp[:])
                nc.vector.tensor_copy(Linv16[:, ic:ic + 1], Linv[:, ic:ic + 1])
                s_i = ic % 4
                for g in range(TC // 4):
                    tp = tps.tile([P, 4, P], dt.float16, tag="tp", name="tp")
                    for s in range(4):
                        jc = g * 4 + s
                        w = nc.tensor.transpose(tp[:, s, :],
                                                Pex[:, jc * P:(jc + 1) * P],
                                                ident16[:])
                        if s == 0:
                            tp_ring.writer(w)
                    # DVE only: keeps the ACT queue clear for the next exp
                    tp_ring.reader(nc.vector.tensor_copy(
                        PT[:, g * 4:(g + 1) * 4, s_i * P:(s_i + 1) * P], tp[:]))
                # broadcast 1/l along the free dim for the block-level scale
                lt = tps.tile([P, 4, P], dt.float16, tag="tp", name="lt")
                tp_ring.writer(nc.tensor.transpose(
                    lt[:, 0, :], Linv16[:, ic:ic + 1].to_broadcast([P, P]),
                    ident16[:]))
                tp_ring.reader(nc.scalar.copy(Lbc[:, ic * P:(ic + 1) * P],
                                              lt[:, 0, :]))

            def stage_pv_chunk(bo, dc, PT):
                zp = zps.tile([P, 512], dt.float32, tag="zp", name="zp")
                for jc in range(TC):
                    w = nc.tensor.matmul(zp[:], V16[:, jc, dc * P:(dc + 1) * P],
                                         PT[:, jc, :],
                                         start=(jc == 0), stop=(jc == TC - 1))
                    if jc == 0:
                        zp_ring.writer(w)
                zr = scr.tile([P, 512], dt.float16, tag="zr", name="zr")
                zp_ring.reader(nc.vector.tensor_tensor(
                    out=zr[:], in0=zp[:],
                    in1=Lbc[:, bo * 512:(bo + 1) * 512], op=ALU.mult))
                nc.scalar.activation(zjunk[:], zr[:], AF.Relu,
                                     bias=linb[:, dc:dc + 1], scale=1.0,
                                     accum_out=zsum[dc][:, bo:bo + 1])

            # software pipeline: scores(ic+1) is emitted after PT(ic-1) and a
            # PV chunk of the previous block, so exp(ic) hides under PE work
            PTs = {}
            stage_scores(0)
            stage_softmax(0)
            for ic in range(TC):
                bo = ic // 4
                if ic % 4 == 0:
                    PTs[bo] = ptp.tile([P, TC, 512], dt.float16, tag="pt", name="pt")
                if ic + 1 < TC:
                    stage_scores(ic + 1)
                    stage_softmax(ic + 1)
                stage_transpose(ic, PTs[bo])
                if bo > 0:
                    stage_pv_chunk(bo - 1, ic % 4, PTs[bo - 1])
                if ic % 4 == 3 and bo > 0:
                    PTs.pop(bo - 1)
            for dc in range(DC):
                stage_pv_chunk(NB - 1, dc, PTs[NB - 1])

            # ---------------- classifier ----------------
            ysum = [scr.tile([P, 1], dt.float32, tag=f"ys{d}", name=f"ys{d}")
                    for d in range(DC)]
            for dc in range(DC):
                nc.vector.tensor_reduce(ysum[dc][:], zsum[dc][:], axis=AX.X,
                                        op=ALU.add)
            op = zps.tile([P, 512], dt.float32, tag="zp", name="optp")
            for dc in range(DC):
                w = nc.tensor.matmul(op[:1, :1], clfw[:, dc:dc + 1], ysum[dc][:],
                                     start=(dc == 0), stop=(dc == DC - 1))
                if dc == 0:
                    zp_ring.writer(w)
            osb = scr.tile([1, 1], dt.float32, tag="osb", name="osb")
            sig = nc.scalar.activation(osb[:], op[:1, :1], AF.Sigmoid,
                                       bias=clfb[:], scale=float(1.0 / T))
            zp_ring.reader(sig)
            carry["prev_end"] = sig
            nc.sync.dma_start(out=out_ap, in_=osb[:])


def _get_nc(iters=1, mm_dtype=None):
    key = (iters,)
    if key not in _COMPILED:
        _COMPILED[key] = _build(iters=iters)
    return _COMPILED[key]


def _in_maps(x, embed, W_q, W_k, W_v, lin_w, lin_b, clf_w, clf_b):
    x = np.ascontiguousarray(np.asarray(x).astype(np.int32))
    common = {
        "embed": np.ascontiguousarray(np.asarray(embed, np.float32)),
        "W_q": np.ascontiguousarray(np.asarray(W_q, np.float32)),
        "W_k": np.ascontiguousarray(np.asarray(W_k, np.float32)),
        "W_v": np.ascontiguousarray(np.asarray(W_v, np.float32)),
        "lin_w": np.ascontiguousarray(np.asarray(lin_w, np.float32)),
        "lin_b": np.ascontiguousarray(np.asarray(lin_b, np.float32).reshape(D)),
        "clf_w": np.ascontiguousarray(np.asarray(clf_w, np.float32).reshape(D)),
        "clf_b": np.ascontiguousarray(np.asarray(clf_b, np.float32).reshape(1)),
    }
    return [dict(common, x=x[c]) for c in range(N_CORES)]


def kernel(x, embed, W_q, W_k, W_v, lin_w, lin_b, clf_w, clf_b):
    from concourse.bass_utils import run_bass_kernel_spmd

    nc = _get_nc()
    in_maps = _in_maps(x, embed, W_q, W_k, W_v, lin_w, lin_b, clf_w, clf_b)
    res = run_bass_kernel_spmd(nc, in_maps, core_ids=list(range(N_CORES)))
    out = np.stack([res.results[c]["out"][0, 0] for c in range(N_CORES)])
    return out.reshape(B, 1).astype(np.float32)
